# revision 8
# baseline (speedup 1.0000x reference)
"""Non-overlapping Conv1d (kernel=2, stride=2) on 8 TRN2 NeuronCores.

out[b, o, p] = sum_{c,k} x[b, c, 2p+k] * w[o, c, k] / sqrt(cin)

Strategy: data-parallel over batch (4 batches per core), weight replicated.
Per batch: out[b] = W0 @ x[b][:, 0::2] + W1 @ x[b][:, 1::2] with the
contraction over cin=128 on the partition dim.  The even/odd deinterleave
happens in the matmul rhs access pattern (stride-2 free dim; measured
same PE throughput as stride-1).  The 1/sqrt(cin) scale is folded into
the weights on the host.

Precision/traffic: the kernel is HBM-bound (input read + output write),
so x is sent as plain fp16 (half the bytes of fp32) and the output is
stored as fp16 and upconverted to fp32 on the host.  End-to-end L2 error
is ~3e-4, far inside the 2e-2 gate.

DMA: x loads alternate across BOTH HWDGE rings (nc.sync / nc.scalar);
output stores ride the gpsimd SWDGE ring.  Stores stay at 0.5 MB (small
stores measured slower per byte); only the final chunk stores
per-512-tile so the pipeline tail after the last x byte is short.
"""

import math
from contextlib import ExitStack

import numpy as np

import concourse.bass as bass
import concourse.mybir as mybir
import concourse.tile as tile
from concourse import bacc
from concourse.bass_utils import run_bass_kernel_spmd

# Problem shape (hardcoded per contract)
BS, CIN, D = 32, 128, 8192
COUT = 128
N_CORES = 8
B_PER_CORE = BS // N_CORES          # 4
P_OUT = D // 2                      # 4096 output positions per (b, o)
PSUM_N = 512                        # fp32 PSUM bank limit = matmul free dim

CHUNK_P = 2048                      # output positions per DMA chunk
N_CHUNKS = P_OUT // CHUNK_P         # per batch
TILES_PER_CHUNK = CHUNK_P // PSUM_N

_cache = {}


def _build():
    nc = bacc.Bacc("TRN2", target_bir_lowering=False, debug=False, num_devices=N_CORES)
    f32 = mybir.dt.float32
    f16 = mybir.dt.float16

    x_d = nc.dram_tensor(
        "xh", [B_PER_CORE, CIN, D], f16, kind="ExternalInput"
    ).ap()
    w_d = nc.dram_tensor("wT", [2, CIN, COUT], f16, kind="ExternalInput").ap()
    out_d = nc.dram_tensor(
        "out", [B_PER_CORE, COUT, P_OUT], f16, kind="ExternalOutput"
    ).ap()

    with tile.TileContext(nc) as tc, ExitStack() as ctx:
        wpool = ctx.enter_context(tc.tile_pool(name="w", bufs=1))
        xpool = ctx.enter_context(tc.tile_pool(name="x", bufs=4))
        opool = ctx.enter_context(tc.tile_pool(name="o", bufs=4))
        ppool = ctx.enter_context(tc.tile_pool(name="p", bufs=8, space="PSUM"))

        # Weights: SBUF [cin, k, cout]; dram layout [k, cin, cout].
        # Loaded first on the ACT HWDGE ring (idle at start, fast ring).
        w_t = wpool.tile([CIN, 2, COUT], f16)
        nc.scalar.dma_start(w_t[:], w_d.rearrange("k c o -> c k o"))

        qi = 0
        for b in range(B_PER_CORE):
            for c in range(N_CHUNKS):
                last = b == B_PER_CORE - 1 and c == N_CHUNKS - 1
                cols = slice(c * 2 * CHUNK_P, (c + 1) * 2 * CHUNK_P)
                x_t = xpool.tile([CIN, CHUNK_P, 2], f16, tag="x")
                xq = nc.sync if qi % 2 == 0 else nc.scalar
                qi += 1
                xq.dma_start(
                    x_t[:], x_d[b, :, cols].rearrange("c (p k) -> c p k", k=2)
                )
                o_t = opool.tile([COUT, CHUNK_P], f16)
                for j in range(TILES_PER_CHUNK):
                    js = slice(j * PSUM_N, (j + 1) * PSUM_N)
                    acc = ppool.tile([COUT, PSUM_N], f32)
                    nc.tensor.matmul(
                        acc[:], w_t[:, 0, :], x_t[:, js, 0], start=True, stop=False
                    )
                    nc.tensor.matmul(
                        acc[:], w_t[:, 1, :], x_t[:, js, 1], start=False, stop=True
                    )
                    nc.vector.tensor_copy(o_t[:, js], acc[:])
                    if last:
                        # per-tile stores so the tail after the final x
                        # byte is one 512-tile deep, not a whole chunk
                        nc.gpsimd.dma_start(
                            out_d[b, :, c * CHUNK_P + j * PSUM_N:
                                  c * CHUNK_P + (j + 1) * PSUM_N],
                            o_t[:, js],
                        )
                if not last:
                    nc.gpsimd.dma_start(
                        out_d[b, :, c * CHUNK_P:(c + 1) * CHUNK_P], o_t[:]
                    )

    nc.compile()
    return nc


def _make_in_maps(x: np.ndarray, weight: np.ndarray) -> list[dict]:
    xh = np.ascontiguousarray(x, dtype=np.float32).astype(np.float16)

    # wT[k, c, o] = weight[o, c, 0, k] / sqrt(cin)
    wT = np.ascontiguousarray(
        np.transpose(weight[:, :, 0, :], (2, 1, 0)) / math.sqrt(CIN), dtype=np.float32
    ).astype(np.float16)

    return [
        {
            "xh": xh[i * B_PER_CORE:(i + 1) * B_PER_CORE],
            "wT": wT,
        }
        for i in range(N_CORES)
    ]


def kernel(x: np.ndarray, weight: np.ndarray) -> np.ndarray:
    if "nc" not in _cache:
        _cache["nc"] = _build()
    nc = _cache["nc"]
    in_maps = _make_in_maps(x, weight)
    res = run_bass_kernel_spmd(nc, in_maps, core_ids=list(range(N_CORES)))
    return np.concatenate(
        [r["out"].astype(np.float32) for r in res.results], axis=0
    )
